# revision 6
# baseline (speedup 1.0000x reference)
"""DMoN forward kernel on 8 TRN2 NeuronCores (Bass/Tile).

  S = softmax(X @ W + b)                       [N,16]  (assignments output)
  c = sum_n S[n];  P = S^T X;  features_pooled = selu(P / c[:,None])
  e2 = sum_e val_e (= 2*n_edges)
  m  = sum_e val_e * S[row_e]        (= S^T degrees)
  t1 = sum_e val_e * <S[row_e], S[col_e]>  (= trace(S^T A S))
  spectral_loss = -(t1 - |m|^2/e2)/e2
  collapse_loss = 0.1*(4*|c|/N - 1)

Nodes row-sharded 12500/core (padded 12544); edges sharded by row-owner,
sorted by col, bucketed into 8 col-chunks (one per owner rank). Per-edge
gathers via GPSIMD ap_gather on SBUF-resident S^T tables; one AllGather for
the global S^T table, one AllReduce for the small stats.
"""
import sys

sys.path.insert(0, "/opt/trn_rl_repo")

import numpy as np

N_CORES = 8
N_NODES = 100000
N_FEAT = 256
N_CLUSTERS = 16
NLOC = 12500
NPAD = 12544            # 98*128
NT = 98
NI = 1024               # ap_gather num_idxs per call
SLOTS_PER_CALL = 8 * NI
CALLS_PER_CHUNK = 8
N_CHUNKS = 8            # one col-chunk per owner rank (12544 nodes each)
SLOTS_PER_CHUNK = CALLS_PER_CHUNK * SLOTS_PER_CALL  # 65536
N_CALLS = N_CHUNKS * CALLS_PER_CHUNK                # 64
E_PAD = N_CHUNKS * SLOTS_PER_CHUNK                  # 524288

SELU_L = 1.0507009873554805
SELU_A = 1.6732632423543772


def _apply_tile_patches():
    import concourse.tile as tile_mod
    from concourse.vector_clock import ScopedClock, VectorClock

    def _patched_drain_and_barrier(self, tick_clock, wait_clock):
        nc = self.nc
        vc = tick_clock.global_clock
        for p in range(len(vc)):
            t = vc[p]
            if t > 0:
                vec = [0] * len(vc)
                vec[p] = t
                nop = nc.sync.nop()
                wait_clock.add_sem_waits(
                    nop.ins, ScopedClock({None: VectorClock(vec)}))
        nc.sync.drain()
        nc.all_engine_barrier()
        assert self.sems is not None
        popped = nc._tile_sem_poison_stack.pop()
        assert popped is self._sem_poison
        nc.clear_and_free_semaphores(list(self.sems.allocated().values()))
        nc.all_engine_barrier()

    tile_mod.TileContext._drain_and_barrier = _patched_drain_and_barrier


_FIX_CTR = [0]


def _fix_sync_waits(nc, max_waits=1):
    import concourse.mybir as mybir

    for bb in nc.m.functions[0].blocks:
        out, changed = [], False
        for inst in bb.instructions:
            si = inst.sync_info
            waits = list(si.on_wait) if si is not None and si.on_wait else []
            if len(waits) > max_waits:
                changed = True
                excess, keep = waits[:-max_waits], waits[-max_waits:]
                for i in range(0, len(excess), max_waits):
                    n = mybir.InstNoOp(
                        name=f"I-waitfix-{_FIX_CTR[0]}", ins=[], outs=[])
                    _FIX_CTR[0] += 1
                    n.engine = inst.engine
                    n.sync_info = mybir.SyncInfo(
                        on_wait=excess[i:i + max_waits], on_update=[])
                    out.append(n)
                inst.sync_info = mybir.SyncInfo(
                    on_wait=keep,
                    on_update=list(si.on_update) if si.on_update else [])
            out.append(inst)
        if changed:
            bb.instructions = out


_NC_CACHE = {}


def _build_program():
    if "nc" in _NC_CACHE:
        return _NC_CACHE["nc"]
    import concourse.bacc as bacc
    import concourse.bass as bass
    import concourse.mybir as mybir
    import concourse.tile as tile

    _apply_tile_patches()

    f32 = mybir.dt.float32
    i32 = mybir.dt.int32
    i16 = mybir.dt.int16
    AP = bass.AP

    nc = bacc.Bacc(None, target_bir_lowering=False)
    nc.num_devices = N_CORES

    xt = nc.dram_tensor("xt", [2, 128, NPAD], f32, kind="ExternalInput")
    xp = nc.dram_tensor("xp", [128, NT, N_FEAT], f32, kind="ExternalInput")
    w_in = nc.dram_tensor("w_in", [N_FEAT, N_CLUSTERS], f32, kind="ExternalInput")
    b_in = nc.dram_tensor("b_in", [1, N_CLUSTERS], f32, kind="ExternalInput")
    maskin = nc.dram_tensor("maskin", [128, NT], f32, kind="ExternalInput")
    idn = nc.dram_tensor("idn", [128, 128], f32, kind="ExternalInput")
    sel = nc.dram_tensor("sel", [128, N_CLUSTERS], f32, kind="ExternalInput")
    onesin = nc.dram_tensor("onesin", [128, 1], f32, kind="ExternalInput")
    onesrow = nc.dram_tensor("onesrow", [1, 128], f32, kind="ExternalInput")
    idxc = nc.dram_tensor("idxc", [N_CALLS, 128, NI // 16], i32, kind="ExternalInput")
    idxr = nc.dram_tensor("idxr", [N_CALLS, 128, NI // 16], i32, kind="ExternalInput")
    valc = nc.dram_tensor("valc", [N_CALLS, 8, NI], f32, kind="ExternalInput")

    out_assign = nc.dram_tensor("out_assign", [NPAD, N_CLUSTERS], f32,
                                kind="ExternalOutput")
    out_feat = nc.dram_tensor("out_feat", [N_CLUSTERS, N_FEAT], f32,
                              kind="ExternalOutput")
    out_loss = nc.dram_tensor("out_loss", [1, 2], f32, kind="ExternalOutput")

    STATS = 4224

    with tile.TileContext(nc) as tc:
        with tc.tile_pool(name="dram", bufs=1, space="DRAM") as dram, \
             tc.tile_pool(name="const", bufs=1) as const:

            cc_slt = dram.tile([16, NPAD], f32)
            ag_out = dram.tile([128, NPAD], f32, addr_space="Shared")
            st_in = dram.tile([1, STATS], f32)
            st_out = dram.tile([1, STATS], f32, addr_space="Shared")

            w_sb = const.tile([128, 2, N_CLUSTERS], f32)
            nc.sync.dma_start(w_sb[:],
                              w_in[:].rearrange("(h p) k -> p h k", p=128))
            b_sb = const.tile([1, N_CLUSTERS], f32)
            nc.sync.dma_start(b_sb[:], b_in[:])
            mask_sb = const.tile([128, NT], f32)
            nc.sync.dma_start(mask_sb[:], maskin[:])
            idn_sb = const.tile([128, 128], f32)
            nc.sync.dma_start(idn_sb[:], idn[:])
            sel_sb = const.tile([128, N_CLUSTERS], f32)
            nc.sync.dma_start(sel_sb[:], sel[:])
            ones_sb = const.tile([128, 1], f32)
            nc.sync.dma_start(ones_sb[:], onesin[:])
            onesr_sb = const.tile([1, 128], f32)
            nc.sync.dma_start(onesr_sb[:], onesrow[:])

            pt_acc = const.tile([128, 2 * N_CLUSTERS], f32)
            nc.vector.memset(pt_acc[:], 0.0)
            c_acc = const.tile([16, 1], f32)
            nc.vector.memset(c_acc[:], 0.0)

            slt_sb = const.tile([16, NPAD], f32)
            accT = const.tile([128, N_CALLS], f32)
            accM = const.tile([128, N_CALLS], f32)
            accE = const.tile([128, N_CALLS], f32)

            # ---------------- phase A ----------------
            NCH, TPC = 7, NT // 7
            with tc.tile_pool(name="pA", bufs=2) as pA, \
                 tc.tile_pool(name="pAs", bufs=3) as pAs, \
                 tc.tile_pool(name="psT", bufs=2, space="PSUM") as psT:
                for ch in range(NCH):
                    n0 = ch * TPC * 128
                    xt_t = pA.tile([128, 2, TPC * 128], f32, tag="xt")
                    nc.sync.dma_start(
                        xt_t[:],
                        xt[:, :, n0:n0 + TPC * 128].rearrange("h p n -> p h n"))
                    xp_t = pA.tile([128, TPC, N_FEAT], f32, tag="xp")
                    nc.sync.dma_start(xp_t[:], xp[:, ch * TPC:(ch + 1) * TPC, :])
                    for t in range(TPC):
                        ti = ch * TPC + t
                        lg = psT.tile([128, N_CLUSTERS], f32, tag="lg")
                        for h in range(2):
                            nc.tensor.matmul(
                                lg[:], xt_t[:, h, t * 128:(t + 1) * 128],
                                w_sb[:, h, :], start=(h == 0), stop=False)
                        nc.tensor.matmul(lg[:], onesr_sb[:], b_sb[:],
                                         start=False, stop=True)
                        ex = pAs.tile([128, N_CLUSTERS], f32, tag="ex")
                        den = pAs.tile([128, 1], f32, tag="den")
                        nc.scalar.activation(ex[:], lg[:],
                                             mybir.ActivationFunctionType.Exp,
                                             accum_out=den[:])
                        rm = pAs.tile([128, 1], f32, tag="rm")
                        nc.vector.reciprocal(rm[:], den[:])
                        nc.vector.tensor_mul(rm[:], rm[:], mask_sb[:, ti:ti + 1])
                        s_t = pAs.tile([128, N_CLUSTERS], f32, tag="s_t")
                        nc.vector.tensor_scalar_mul(s_t[:], ex[:], rm[:])
                        nc.sync.dma_start(
                            out_assign[ti * 128:(ti + 1) * 128, :], s_t[:])
                        stp = psT.tile([16, 128], f32, tag="stp")
                        nc.tensor.transpose(stp[:], s_t[:], idn_sb[:])
                        nc.vector.tensor_copy(
                            slt_sb[:, ti * 128:(ti + 1) * 128], stp[:])
                        ptt = psT.tile([128, 2 * N_CLUSTERS], f32, tag="ptt")
                        for h in range(2):
                            nc.tensor.matmul(
                                ptt[:, h * N_CLUSTERS:(h + 1) * N_CLUSTERS],
                                xp_t[:, t, h * 128:(h + 1) * 128], s_t[:],
                                start=True, stop=True)
                        nc.vector.tensor_add(pt_acc[:], pt_acc[:], ptt[:])
                        ctt = psT.tile([16, 1], f32, tag="ctt")
                        nc.tensor.matmul(ctt[:], s_t[:], ones_sb[:],
                                         start=True, stop=True)
                        nc.vector.tensor_add(c_acc[:], c_acc[:], ctt[:])

            nc.sync.dma_start(cc_slt[:], slt_sb[:])
            nc.gpsimd.collective_compute(
                "AllGather", mybir.AluOpType.bypass,
                replica_groups=[list(range(N_CORES))],
                ins=[cc_slt[:]], outs=[ag_out[:]])

            # local-row table, replicated to all 8 q7 groups
            rt_sb = const.tile([128, NPAD], f32)
            nc.sync.dma_start(
                rt_sb[:],
                AP(cc_slt.tensor, cc_slt.offset,
                   [[0, 8], [NPAD, 16], [1, NPAD]]))

            # ---------------- edge phase ----------------
            with tc.tile_pool(name="pC", bufs=1) as pC, \
                 tc.tile_pool(name="pG", bufs=2) as pG:
                for chunk in range(N_CHUNKS):
                    ct_sb = pC.tile([128, NPAD], f32, tag="ct")
                    nc.sync.dma_start(
                        ct_sb[:],
                        AP(ag_out.tensor, ag_out.offset + 16 * chunk * NPAD,
                           [[0, 8], [NPAD, 16], [1, NPAD]]))
                    for s in range(CALLS_PER_CHUNK):
                        call = chunk * CALLS_PER_CHUNK + s
                        ic32 = pG.tile([128, NI // 16], i32, tag="ic32")
                        nc.sync.dma_start(ic32[:], idxc[call])
                        ic16 = pG.tile([128, NI // 16], i16, tag="ic16")
                        nc.vector.tensor_copy(ic16[:], ic32[:])
                        ir32 = pG.tile([128, NI // 16], i32, tag="ir32")
                        nc.sync.dma_start(ir32[:], idxr[call])
                        ir16 = pG.tile([128, NI // 16], i16, tag="ir16")
                        nc.vector.tensor_copy(ir16[:], ir32[:])
                        vr = pG.tile([128, NI], f32, tag="vr")
                        nc.sync.dma_start(
                            vr[:],
                            AP(valc[:].tensor, valc[:].offset + call * 8 * NI,
                               [[NI, 8], [0, 16], [1, NI]]))
                        gc = pG.tile([128, NI], f32, tag="gc")
                        nc.gpsimd.ap_gather(gc[:], ct_sb[:], ic16[:],
                                            channels=128, num_elems=NPAD,
                                            d=1, num_idxs=NI)
                        gr = pG.tile([128, NI], f32, tag="gr")
                        nc.gpsimd.ap_gather(gr[:], rt_sb[:], ir16[:],
                                            channels=128, num_elems=NPAD,
                                            d=1, num_idxs=NI)
                        grv = pG.tile([128, NI], f32, tag="grv")
                        nc.vector.tensor_mul(grv[:], gr[:], vr[:])
                        nc.vector.tensor_reduce(
                            accM[:, call:call + 1], grv[:],
                            axis=mybir.AxisListType.X, op=mybir.AluOpType.add)
                        nc.vector.tensor_mul(grv[:], grv[:], gc[:])
                        nc.vector.tensor_reduce(
                            accT[:, call:call + 1], grv[:],
                            axis=mybir.AxisListType.X, op=mybir.AluOpType.add)
                        nc.vector.tensor_reduce(
                            accE[:, call:call + 1], vr[:],
                            axis=mybir.AxisListType.X, op=mybir.AluOpType.add)

            # ---------------- reduce + final ----------------
            with tc.tile_pool(name="pF", bufs=1) as pF, \
                 tc.tile_pool(name="psF", bufs=1, space="PSUM") as psF:
                parts = pF.tile([128, 2], f32)
                nc.vector.tensor_reduce(parts[:, 0:1], accT[:],
                                        axis=mybir.AxisListType.X,
                                        op=mybir.AluOpType.add)
                nc.vector.tensor_reduce(parts[:, 1:2], accE[:],
                                        axis=mybir.AxisListType.X,
                                        op=mybir.AluOpType.add)
                mparts = pF.tile([128, 1], f32)
                nc.vector.tensor_reduce(mparts[:], accM[:],
                                        axis=mybir.AxisListType.X,
                                        op=mybir.AluOpType.add)
                te_ps = psF.tile([1, 2], f32)
                nc.tensor.matmul(te_ps[:], ones_sb[:], parts[:],
                                 start=True, stop=True)
                m_ps = psF.tile([16, 1], f32)
                nc.tensor.matmul(m_ps[:], sel_sb[:], mparts[:],
                                 start=True, stop=True)

                pt_sb = pt_acc
                c_sb0 = c_acc
                te_sb = pF.tile([1, 2], f32)
                nc.vector.tensor_copy(te_sb[:], te_ps[:])
                nc.vector.tensor_scalar_mul(te_sb[:, 1:2], te_sb[:, 1:2],
                                            1.0 / 16.0)
                m_sb0 = pF.tile([16, 1], f32)
                nc.vector.tensor_copy(m_sb0[:], m_ps[:])

                stt, sto = st_in.tensor, st_out.tensor
                nc.sync.dma_start(
                    AP(stt, st_in.offset, [[32, 128], [1, 32]]), pt_sb[:])
                nc.sync.dma_start(
                    AP(stt, st_in.offset + 4096, [[1, 16], [1, 1]]), c_sb0[:])
                nc.sync.dma_start(
                    AP(stt, st_in.offset + 4112, [[1, 16], [1, 1]]), m_sb0[:])
                nc.sync.dma_start(
                    AP(stt, st_in.offset + 4128, [[2, 1], [1, 2]]), te_sb[:])
                nc.gpsimd.collective_compute(
                    "AllReduce", mybir.AluOpType.add,
                    replica_groups=[list(range(N_CORES))],
                    ins=[st_in[:]], outs=[st_out[:]])

                ptr = pF.tile([128, 2 * N_CLUSTERS], f32)
                nc.sync.dma_start(
                    ptr[:], AP(sto, st_out.offset, [[32, 128], [1, 32]]))
                c_red = pF.tile([16, 1], f32)
                nc.sync.dma_start(
                    c_red[:], AP(sto, st_out.offset + 4096, [[1, 16], [1, 1]]))
                flat = pF.tile([1, 34], f32)
                nc.sync.dma_start(
                    flat[:], AP(sto, st_out.offset + 4096, [[34, 1], [1, 34]]))

                fp_ps = psF.tile([16, N_FEAT], f32)
                for h in range(2):
                    nc.tensor.transpose(
                        fp_ps[:, h * 128:(h + 1) * 128],
                        ptr[:, h * N_CLUSTERS:(h + 1) * N_CLUSTERS], idn_sb[:])
                rc = pF.tile([16, 1], f32)
                nc.vector.reciprocal(rc[:], c_red[:])
                fpd = pF.tile([16, N_FEAT], f32)
                nc.vector.tensor_scalar_mul(fpd[:], fp_ps[:], rc[:])
                pos = pF.tile([16, N_FEAT], f32)
                nc.vector.tensor_scalar_max(pos[:], fpd[:], 0.0)
                neg = pF.tile([16, N_FEAT], f32)
                nc.vector.tensor_scalar_min(neg[:], fpd[:], 0.0)
                en = pF.tile([16, N_FEAT], f32)
                nc.scalar.activation(en[:], neg[:],
                                     mybir.ActivationFunctionType.Exp)
                nc.vector.tensor_scalar(en[:], en[:], -1.0, SELU_L * SELU_A,
                                        op0=mybir.AluOpType.add,
                                        op1=mybir.AluOpType.mult)
                nc.vector.tensor_scalar_mul(pos[:], pos[:], SELU_L)
                nc.vector.tensor_add(pos[:], pos[:], en[:])
                nc.sync.dma_start(out_feat[:], pos[:])

                sq = pF.tile([1, 34], f32)
                nc.vector.tensor_mul(sq[:], flat[:], flat[:])
                cn = pF.tile([1, 4], f32)
                nc.vector.tensor_reduce(cn[:, 0:1], sq[:, 0:16],
                                        axis=mybir.AxisListType.X,
                                        op=mybir.AluOpType.add)
                nc.vector.tensor_reduce(cn[:, 1:2], sq[:, 16:32],
                                        axis=mybir.AxisListType.X,
                                        op=mybir.AluOpType.add)
                nc.vector.reciprocal(cn[:, 2:3], flat[:, 33:34])
                ls = pF.tile([1, 2], f32)
                nc.vector.tensor_mul(ls[:, 0:1], cn[:, 1:2], cn[:, 2:3])
                nc.vector.tensor_tensor(out=ls[:, 0:1], in0=ls[:, 0:1],
                                        in1=flat[:, 32:33],
                                        op=mybir.AluOpType.subtract)
                nc.vector.tensor_mul(ls[:, 0:1], ls[:, 0:1], cn[:, 2:3])
                nc.scalar.activation(cn[:, 3:4], cn[:, 0:1],
                                     mybir.ActivationFunctionType.Sqrt)
                nc.vector.tensor_scalar(
                    ls[:, 1:2], cn[:, 3:4],
                    float(np.sqrt(N_CLUSTERS)) / N_NODES, -1.0,
                    op0=mybir.AluOpType.mult, op1=mybir.AluOpType.add)
                nc.vector.tensor_scalar_mul(ls[:, 1:2], ls[:, 1:2], 0.1)
                nc.sync.dma_start(out_loss[:], ls[:])

    nc.finalize()
    _fix_sync_waits(nc)
    _NC_CACHE["nc"] = nc
    return nc


def _wrap16(idx_grp):
    out = np.zeros((128, NI // 16), dtype=np.int32)
    for g in range(8):
        out[g * 16:(g + 1) * 16, :] = idx_grp[g].reshape(NI // 16, 16).T
    return out


def kernel(features, W, b, edge_row, edge_col, edge_val):
    features = np.asarray(features, dtype=np.float32)
    W = np.asarray(W, dtype=np.float32)
    b = np.asarray(b, dtype=np.float32)
    edge_row = np.asarray(edge_row, dtype=np.int32)
    edge_col = np.asarray(edge_col, dtype=np.int32)
    edge_val = np.asarray(edge_val, dtype=np.float32)

    from concourse.bass_utils import run_bass_kernel_spmd

    nc = _build_program()

    idn = np.eye(128, dtype=np.float32)
    sel = np.zeros((128, 16), dtype=np.float32)
    for g in range(8):
        sel[g * 16:(g + 1) * 16] = np.eye(16, dtype=np.float32)
    ones = np.ones((128, 1), dtype=np.float32)
    onesr = np.ones((1, 128), dtype=np.float32)
    bq = b.reshape(1, 16).astype(np.float32)

    owner = edge_row // NLOC
    in_maps = []
    for m in range(N_CORES):
        X = np.zeros((NPAD, N_FEAT), dtype=np.float32)
        X[:NLOC] = features[m * NLOC:(m + 1) * NLOC]
        xt = np.ascontiguousarray(X.T.reshape(2, 128, NPAD))
        xp = np.ascontiguousarray(X.reshape(NT, 128, N_FEAT).transpose(1, 0, 2))
        mask = np.zeros(NPAD, dtype=np.float32)
        mask[:NLOC] = 1.0
        mask = np.ascontiguousarray(mask.reshape(NT, 128).T)

        e = np.nonzero(owner == m)[0]
        cols = edge_col[e]
        order = np.argsort(cols, kind="stable")
        e = e[order]
        cols = cols[order]
        rows_l = (edge_row[e] - m * NLOC).astype(np.int32)
        vals = edge_val[e]
        chunk_of = cols // NLOC
        col_rel = (cols % NLOC).astype(np.int32)

        idxc_a = np.zeros((N_CALLS, 8, NI), dtype=np.int32)
        idxr_a = np.zeros((N_CALLS, 8, NI), dtype=np.int32)
        valc_a = np.zeros((N_CALLS, 8, NI), dtype=np.float32)
        for c in range(N_CHUNKS):
            ec = np.nonzero(chunk_of == c)[0]
            n_ec = ec.size
            assert n_ec <= SLOTS_PER_CHUNK, (m, c, n_ec)
            ccol = np.zeros(SLOTS_PER_CHUNK, dtype=np.int32)
            crow = np.zeros(SLOTS_PER_CHUNK, dtype=np.int32)
            cval = np.zeros(SLOTS_PER_CHUNK, dtype=np.float32)
            ccol[:n_ec] = col_rel[ec]
            crow[:n_ec] = rows_l[ec]
            cval[:n_ec] = vals[ec]
            base = c * CALLS_PER_CHUNK
            idxc_a[base:base + CALLS_PER_CHUNK] = ccol.reshape(
                CALLS_PER_CHUNK, 8, NI)
            idxr_a[base:base + CALLS_PER_CHUNK] = crow.reshape(
                CALLS_PER_CHUNK, 8, NI)
            valc_a[base:base + CALLS_PER_CHUNK] = cval.reshape(
                CALLS_PER_CHUNK, 8, NI)
        idxc_w = np.stack([_wrap16(idxc_a[k]) for k in range(N_CALLS)])
        idxr_w = np.stack([_wrap16(idxr_a[k]) for k in range(N_CALLS)])

        in_maps.append({
            "xt": xt, "xp": xp, "w_in": W, "b_in": bq, "maskin": mask,
            "idn": idn, "sel": sel, "onesin": ones, "onesrow": onesr,
            "idxc": idxc_w, "idxr": idxr_w, "valc": valc_a,
        })

    res = run_bass_kernel_spmd(nc, in_maps, core_ids=list(range(N_CORES)))

    assigns = np.concatenate(
        [res.results[m]["out_assign"][:NLOC] for m in range(N_CORES)], axis=0)
    fp = res.results[0]["out_feat"]
    losses = res.results[0]["out_loss"]
    return (fp, assigns, np.float32(losses[0, 0]), np.float32(losses[0, 1]))


# revision 7
# speedup vs baseline: 8.9158x; 8.9158x over previous
"""DMoN forward kernel on 8 TRN2 NeuronCores (Bass/Tile).

  S = softmax(X @ W + b)                       [N,16]  (assignments output)
  c = sum_n S[n];  P = S^T X;  features_pooled = selu(P / c[:,None])
  e2 = sum_e val_e (= 2*n_edges)
  m  = sum_e val_e * S[row_e]        (= S^T degrees)
  t1 = sum_e val_e * <S[row_e], S[col_e]>  (= trace(S^T A S))
  spectral_loss = -(t1 - |m|^2/e2)/e2
  collapse_loss = 0.1*(4*|c|/N - 1)

Nodes row-sharded 12500/core (padded 12544); edges sharded by row-owner,
sorted by col, bucketed into 8 col-chunks (one per owner rank). Per-edge
gathers via GPSIMD ap_gather on SBUF-resident S^T tables; one AllGather for
the global S^T table, one AllReduce for the small stats.
"""
import sys

sys.path.insert(0, "/opt/trn_rl_repo")

import numpy as np

N_CORES = 8
N_NODES = 100000
N_FEAT = 256
N_CLUSTERS = 16
NLOC = 12500
NPAD = 12544            # 98*128
NT = 98
NI = 1024               # ap_gather num_idxs per call
SLOTS_PER_CALL = 8 * NI
CALLS_PER_CHUNK = 7
N_CHUNKS = 8            # one col-chunk per owner rank (12544 nodes each)
SLOTS_PER_CHUNK = CALLS_PER_CHUNK * SLOTS_PER_CALL  # 57344
N_CALLS = N_CHUNKS * CALLS_PER_CHUNK                # 56
E_PAD = N_CHUNKS * SLOTS_PER_CHUNK                  # 458752

SELU_L = 1.0507009873554805
SELU_A = 1.6732632423543772


def _apply_tile_patches():
    import concourse.tile as tile_mod
    from concourse.vector_clock import ScopedClock, VectorClock

    def _patched_drain_and_barrier(self, tick_clock, wait_clock):
        nc = self.nc
        vc = tick_clock.global_clock
        for p in range(len(vc)):
            t = vc[p]
            if t > 0:
                vec = [0] * len(vc)
                vec[p] = t
                nop = nc.sync.nop()
                wait_clock.add_sem_waits(
                    nop.ins, ScopedClock({None: VectorClock(vec)}))
        nc.sync.drain()
        nc.all_engine_barrier()
        assert self.sems is not None
        popped = nc._tile_sem_poison_stack.pop()
        assert popped is self._sem_poison
        nc.clear_and_free_semaphores(list(self.sems.allocated().values()))
        nc.all_engine_barrier()

    tile_mod.TileContext._drain_and_barrier = _patched_drain_and_barrier


_FIX_CTR = [0]


def _fix_sync_waits(nc, max_waits=1):
    import concourse.mybir as mybir

    for bb in nc.m.functions[0].blocks:
        out, changed = [], False
        for inst in bb.instructions:
            si = inst.sync_info
            waits = list(si.on_wait) if si is not None and si.on_wait else []
            if len(waits) > max_waits:
                changed = True
                excess, keep = waits[:-max_waits], waits[-max_waits:]
                for i in range(0, len(excess), max_waits):
                    n = mybir.InstNoOp(
                        name=f"I-waitfix-{_FIX_CTR[0]}", ins=[], outs=[])
                    _FIX_CTR[0] += 1
                    n.engine = inst.engine
                    n.sync_info = mybir.SyncInfo(
                        on_wait=excess[i:i + max_waits], on_update=[])
                    out.append(n)
                inst.sync_info = mybir.SyncInfo(
                    on_wait=keep,
                    on_update=list(si.on_update) if si.on_update else [])
            out.append(inst)
        if changed:
            bb.instructions = out


_NC_CACHE = {}


def _build_program():
    if "nc" in _NC_CACHE:
        return _NC_CACHE["nc"]
    import concourse.bacc as bacc
    import concourse.bass as bass
    import concourse.mybir as mybir
    import concourse.tile as tile

    _apply_tile_patches()

    f32 = mybir.dt.float32
    i32 = mybir.dt.int32
    i16 = mybir.dt.int16
    AP = bass.AP

    nc = bacc.Bacc(None, target_bir_lowering=False)
    nc.num_devices = N_CORES

    xt = nc.dram_tensor("xt", [2, 128, NPAD], f32, kind="ExternalInput")
    xp = nc.dram_tensor("xp", [128, NT, N_FEAT], f32, kind="ExternalInput")
    w_in = nc.dram_tensor("w_in", [N_FEAT, N_CLUSTERS], f32, kind="ExternalInput")
    b_in = nc.dram_tensor("b_in", [1, N_CLUSTERS], f32, kind="ExternalInput")
    maskin = nc.dram_tensor("maskin", [128, NT], f32, kind="ExternalInput")
    idn = nc.dram_tensor("idn", [128, 128], f32, kind="ExternalInput")
    sel = nc.dram_tensor("sel", [128, N_CLUSTERS], f32, kind="ExternalInput")
    onesin = nc.dram_tensor("onesin", [128, 1], f32, kind="ExternalInput")
    onesrow = nc.dram_tensor("onesrow", [1, 128], f32, kind="ExternalInput")
    idxc = nc.dram_tensor("idxc", [N_CALLS, 128, NI // 16], i32, kind="ExternalInput")
    idxr = nc.dram_tensor("idxr", [N_CALLS, 128, NI // 16], i32, kind="ExternalInput")
    valc = nc.dram_tensor("valc", [N_CALLS, 8, NI], f32, kind="ExternalInput")

    out_assign = nc.dram_tensor("out_assign", [NPAD, N_CLUSTERS], f32,
                                kind="ExternalOutput")
    out_feat = nc.dram_tensor("out_feat", [N_CLUSTERS, N_FEAT], f32,
                              kind="ExternalOutput")
    out_loss = nc.dram_tensor("out_loss", [1, 2], f32, kind="ExternalOutput")

    STATS = 4224

    with tile.TileContext(nc) as tc:
        with tc.tile_pool(name="dram", bufs=1, space="DRAM") as dram, \
             tc.tile_pool(name="const", bufs=1) as const:

            cc_slt = dram.tile([16, NPAD], f32)
            ag_out = dram.tile([128, NPAD], f32, addr_space="Shared")
            st_in = dram.tile([1, STATS], f32)
            st_out = dram.tile([1, STATS], f32, addr_space="Shared")

            w_sb = const.tile([128, 2, N_CLUSTERS], f32)
            nc.sync.dma_start(w_sb[:],
                              w_in[:].rearrange("(h p) k -> p h k", p=128))
            b_sb = const.tile([1, N_CLUSTERS], f32)
            nc.sync.dma_start(b_sb[:], b_in[:])
            mask_sb = const.tile([128, NT], f32)
            nc.sync.dma_start(mask_sb[:], maskin[:])
            idn_sb = const.tile([128, 128], f32)
            nc.sync.dma_start(idn_sb[:], idn[:])
            sel_sb = const.tile([128, N_CLUSTERS], f32)
            nc.sync.dma_start(sel_sb[:], sel[:])
            ones_sb = const.tile([128, 1], f32)
            nc.sync.dma_start(ones_sb[:], onesin[:])
            onesr_sb = const.tile([1, 128], f32)
            nc.sync.dma_start(onesr_sb[:], onesrow[:])

            pt_acc = const.tile([128, 2 * N_CLUSTERS], f32)
            nc.vector.memset(pt_acc[:], 0.0)
            c_acc = const.tile([16, 1], f32)
            nc.vector.memset(c_acc[:], 0.0)

            accT = const.tile([128, N_CALLS], f32)
            accM = const.tile([128, N_CALLS], f32)
            accE = const.tile([128, N_CALLS], f32)

            # ---------------- phase A ----------------
            NCH, TPC = 7, NT // 7
            with tc.tile_pool(name="pA", bufs=2) as pA, \
                 tc.tile_pool(name="pAs", bufs=3) as pAs, \
                 tc.tile_pool(name="psT", bufs=2, space="PSUM") as psT:
                slt_sb = pA.tile([16, NPAD], f32, tag="slt", bufs=1)
                for ch in range(NCH):
                    n0 = ch * TPC * 128
                    xt_t = pA.tile([128, 2, TPC * 128], f32, tag="xt")
                    nc.sync.dma_start(
                        xt_t[:],
                        xt[:, :, n0:n0 + TPC * 128].rearrange("h p n -> p h n"))
                    xp_t = pA.tile([128, TPC, N_FEAT], f32, tag="xp")
                    nc.sync.dma_start(xp_t[:], xp[:, ch * TPC:(ch + 1) * TPC, :])
                    for t in range(TPC):
                        ti = ch * TPC + t
                        lg = psT.tile([128, N_CLUSTERS], f32, tag="lg")
                        for h in range(2):
                            nc.tensor.matmul(
                                lg[:], xt_t[:, h, t * 128:(t + 1) * 128],
                                w_sb[:, h, :], start=(h == 0), stop=False)
                        nc.tensor.matmul(lg[:], onesr_sb[:], b_sb[:],
                                         start=False, stop=True)
                        ex = pAs.tile([128, N_CLUSTERS], f32, tag="ex")
                        den = pAs.tile([128, 1], f32, tag="den")
                        nc.scalar.activation(ex[:], lg[:],
                                             mybir.ActivationFunctionType.Exp,
                                             accum_out=den[:])
                        rm = pAs.tile([128, 1], f32, tag="rm")
                        nc.vector.reciprocal(rm[:], den[:])
                        nc.vector.tensor_mul(rm[:], rm[:], mask_sb[:, ti:ti + 1])
                        s_t = pAs.tile([128, N_CLUSTERS], f32, tag="s_t")
                        nc.vector.tensor_scalar_mul(s_t[:], ex[:], rm[:])
                        nc.sync.dma_start(
                            out_assign[ti * 128:(ti + 1) * 128, :], s_t[:])
                        stp = psT.tile([16, 128], f32, tag="stp")
                        nc.tensor.transpose(stp[:], s_t[:], idn_sb[:])
                        nc.vector.tensor_copy(
                            slt_sb[:, ti * 128:(ti + 1) * 128], stp[:])
                        ptt = psT.tile([128, 2 * N_CLUSTERS], f32, tag="ptt")
                        for h in range(2):
                            nc.tensor.matmul(
                                ptt[:, h * N_CLUSTERS:(h + 1) * N_CLUSTERS],
                                xp_t[:, t, h * 128:(h + 1) * 128], s_t[:],
                                start=True, stop=True)
                        nc.vector.tensor_add(pt_acc[:], pt_acc[:], ptt[:])
                        ctt = psT.tile([16, 1], f32, tag="ctt")
                        nc.tensor.matmul(ctt[:], s_t[:], ones_sb[:],
                                         start=True, stop=True)
                        nc.vector.tensor_add(c_acc[:], c_acc[:], ctt[:])
                nc.sync.dma_start(cc_slt[:], slt_sb[:])

            nc.gpsimd.collective_compute(
                "AllGather", mybir.AluOpType.bypass,
                replica_groups=[list(range(N_CORES))],
                ins=[cc_slt[:]], outs=[ag_out[:]])

            # local-row table, replicated to all 8 q7 groups
            rt_sb = const.tile([128, NPAD], f32)
            nc.sync.dma_start(
                rt_sb[:],
                AP(cc_slt.tensor, cc_slt.offset,
                   [[0, 8], [NPAD, 16], [1, NPAD]]))

            # ---------------- edge phase ----------------
            with tc.tile_pool(name="pC", bufs=1) as pC, \
                 tc.tile_pool(name="pG", bufs=2) as pG:
                for chunk in range(N_CHUNKS):
                    ct_sb = pC.tile([128, NPAD], f32, tag="ct")
                    nc.sync.dma_start(
                        ct_sb[:],
                        AP(ag_out.tensor, ag_out.offset + 16 * chunk * NPAD,
                           [[0, 8], [NPAD, 16], [1, NPAD]]))
                    for s in range(CALLS_PER_CHUNK):
                        call = chunk * CALLS_PER_CHUNK + s
                        ic32 = pG.tile([128, NI // 16], i32, tag="ic32")
                        nc.sync.dma_start(ic32[:], idxc[call])
                        ic16 = pG.tile([128, NI // 16], i16, tag="ic16")
                        nc.vector.tensor_copy(ic16[:], ic32[:])
                        ir32 = pG.tile([128, NI // 16], i32, tag="ir32")
                        nc.sync.dma_start(ir32[:], idxr[call])
                        ir16 = pG.tile([128, NI // 16], i16, tag="ir16")
                        nc.vector.tensor_copy(ir16[:], ir32[:])
                        vr = pG.tile([128, NI], f32, tag="vr")
                        nc.sync.dma_start(
                            vr[:],
                            AP(valc[:].tensor, valc[:].offset + call * 8 * NI,
                               [[NI, 8], [0, 16], [1, NI]]))
                        gc = pG.tile([128, NI], f32, tag="gc")
                        nc.gpsimd.ap_gather(gc[:], ct_sb[:], ic16[:],
                                            channels=128, num_elems=NPAD,
                                            d=1, num_idxs=NI)
                        gr = pG.tile([128, NI], f32, tag="gr")
                        nc.gpsimd.ap_gather(gr[:], rt_sb[:], ir16[:],
                                            channels=128, num_elems=NPAD,
                                            d=1, num_idxs=NI)
                        grv = pG.tile([128, NI], f32, tag="grv")
                        nc.vector.tensor_mul(grv[:], gr[:], vr[:])
                        nc.vector.tensor_reduce(
                            accM[:, call:call + 1], grv[:],
                            axis=mybir.AxisListType.X, op=mybir.AluOpType.add)
                        nc.vector.tensor_mul(grv[:], grv[:], gc[:])
                        nc.vector.tensor_reduce(
                            accT[:, call:call + 1], grv[:],
                            axis=mybir.AxisListType.X, op=mybir.AluOpType.add)
                        nc.vector.tensor_reduce(
                            accE[:, call:call + 1], vr[:],
                            axis=mybir.AxisListType.X, op=mybir.AluOpType.add)

            # ---------------- reduce + final ----------------
            with tc.tile_pool(name="pF", bufs=1) as pF, \
                 tc.tile_pool(name="psF", bufs=1, space="PSUM") as psF:
                parts = pF.tile([128, 2], f32)
                nc.vector.tensor_reduce(parts[:, 0:1], accT[:],
                                        axis=mybir.AxisListType.X,
                                        op=mybir.AluOpType.add)
                nc.vector.tensor_reduce(parts[:, 1:2], accE[:],
                                        axis=mybir.AxisListType.X,
                                        op=mybir.AluOpType.add)
                mparts = pF.tile([128, 1], f32)
                nc.vector.tensor_reduce(mparts[:], accM[:],
                                        axis=mybir.AxisListType.X,
                                        op=mybir.AluOpType.add)
                te_ps = psF.tile([1, 2], f32)
                nc.tensor.matmul(te_ps[:], ones_sb[:], parts[:],
                                 start=True, stop=True)
                m_ps = psF.tile([16, 1], f32)
                nc.tensor.matmul(m_ps[:], sel_sb[:], mparts[:],
                                 start=True, stop=True)

                pt_sb = pt_acc
                c_sb0 = c_acc
                te_sb = pF.tile([1, 2], f32)
                nc.vector.tensor_copy(te_sb[:], te_ps[:])
                nc.vector.tensor_scalar_mul(te_sb[:, 1:2], te_sb[:, 1:2],
                                            1.0 / 16.0)
                m_sb0 = pF.tile([16, 1], f32)
                nc.vector.tensor_copy(m_sb0[:], m_ps[:])

                stt, sto = st_in.tensor, st_out.tensor
                nc.sync.dma_start(
                    AP(stt, st_in.offset, [[32, 128], [1, 32]]), pt_sb[:])
                nc.sync.dma_start(
                    AP(stt, st_in.offset + 4096, [[1, 16], [1, 1]]), c_sb0[:])
                nc.sync.dma_start(
                    AP(stt, st_in.offset + 4112, [[1, 16], [1, 1]]), m_sb0[:])
                nc.sync.dma_start(
                    AP(stt, st_in.offset + 4128, [[2, 1], [1, 2]]), te_sb[:])
                nc.gpsimd.collective_compute(
                    "AllReduce", mybir.AluOpType.add,
                    replica_groups=[list(range(N_CORES))],
                    ins=[st_in[:]], outs=[st_out[:]])

                ptr = pF.tile([128, 2 * N_CLUSTERS], f32)
                nc.sync.dma_start(
                    ptr[:], AP(sto, st_out.offset, [[32, 128], [1, 32]]))
                c_red = pF.tile([16, 1], f32)
                nc.sync.dma_start(
                    c_red[:], AP(sto, st_out.offset + 4096, [[1, 16], [1, 1]]))
                flat = pF.tile([1, 34], f32)
                nc.sync.dma_start(
                    flat[:], AP(sto, st_out.offset + 4096, [[34, 1], [1, 34]]))

                fp_ps = psF.tile([16, N_FEAT], f32)
                for h in range(2):
                    nc.tensor.transpose(
                        fp_ps[:, h * 128:(h + 1) * 128],
                        ptr[:, h * N_CLUSTERS:(h + 1) * N_CLUSTERS], idn_sb[:])
                rc = pF.tile([16, 1], f32)
                nc.vector.reciprocal(rc[:], c_red[:])
                fpd = pF.tile([16, N_FEAT], f32)
                nc.vector.tensor_scalar_mul(fpd[:], fp_ps[:], rc[:])
                pos = pF.tile([16, N_FEAT], f32)
                nc.vector.tensor_scalar_max(pos[:], fpd[:], 0.0)
                neg = pF.tile([16, N_FEAT], f32)
                nc.vector.tensor_scalar_min(neg[:], fpd[:], 0.0)
                en = pF.tile([16, N_FEAT], f32)
                nc.scalar.activation(en[:], neg[:],
                                     mybir.ActivationFunctionType.Exp)
                nc.vector.tensor_scalar(en[:], en[:], -1.0, SELU_L * SELU_A,
                                        op0=mybir.AluOpType.add,
                                        op1=mybir.AluOpType.mult)
                nc.vector.tensor_scalar_mul(pos[:], pos[:], SELU_L)
                nc.vector.tensor_add(pos[:], pos[:], en[:])
                nc.sync.dma_start(out_feat[:], pos[:])

                sq = pF.tile([1, 34], f32)
                nc.vector.tensor_mul(sq[:], flat[:], flat[:])
                cn = pF.tile([1, 4], f32)
                nc.vector.tensor_reduce(cn[:, 0:1], sq[:, 0:16],
                                        axis=mybir.AxisListType.X,
                                        op=mybir.AluOpType.add)
                nc.vector.tensor_reduce(cn[:, 1:2], sq[:, 16:32],
                                        axis=mybir.AxisListType.X,
                                        op=mybir.AluOpType.add)
                nc.vector.reciprocal(cn[:, 2:3], flat[:, 33:34])
                ls = pF.tile([1, 2], f32)
                nc.vector.tensor_mul(ls[:, 0:1], cn[:, 1:2], cn[:, 2:3])
                nc.vector.tensor_tensor(out=ls[:, 0:1], in0=ls[:, 0:1],
                                        in1=flat[:, 32:33],
                                        op=mybir.AluOpType.subtract)
                nc.vector.tensor_mul(ls[:, 0:1], ls[:, 0:1], cn[:, 2:3])
                nc.scalar.activation(cn[:, 3:4], cn[:, 0:1],
                                     mybir.ActivationFunctionType.Sqrt)
                nc.vector.tensor_scalar(
                    ls[:, 1:2], cn[:, 3:4],
                    float(np.sqrt(N_CLUSTERS)) / N_NODES, -1.0,
                    op0=mybir.AluOpType.mult, op1=mybir.AluOpType.add)
                nc.vector.tensor_scalar_mul(ls[:, 1:2], ls[:, 1:2], 0.1)
                nc.sync.dma_start(out_loss[:], ls[:])

    nc.finalize()
    _fix_sync_waits(nc)
    _NC_CACHE["nc"] = nc
    return nc


def _wrap16(idx_grp):
    out = np.zeros((128, NI // 16), dtype=np.int32)
    for g in range(8):
        out[g * 16:(g + 1) * 16, :] = idx_grp[g].reshape(NI // 16, 16).T
    return out


def kernel(features, W, b, edge_row, edge_col, edge_val):
    features = np.asarray(features, dtype=np.float32)
    W = np.asarray(W, dtype=np.float32)
    b = np.asarray(b, dtype=np.float32)
    edge_row = np.asarray(edge_row, dtype=np.int32)
    edge_col = np.asarray(edge_col, dtype=np.int32)
    edge_val = np.asarray(edge_val, dtype=np.float32)

    from concourse.bass_utils import run_bass_kernel_spmd

    nc = _build_program()

    idn = np.eye(128, dtype=np.float32)
    sel = np.zeros((128, 16), dtype=np.float32)
    for g in range(8):
        sel[g * 16:(g + 1) * 16] = np.eye(16, dtype=np.float32)
    ones = np.ones((128, 1), dtype=np.float32)
    onesr = np.ones((1, 128), dtype=np.float32)
    bq = b.reshape(1, 16).astype(np.float32)

    owner = edge_row // NLOC
    in_maps = []
    for m in range(N_CORES):
        X = np.zeros((NPAD, N_FEAT), dtype=np.float32)
        X[:NLOC] = features[m * NLOC:(m + 1) * NLOC]
        xt = np.ascontiguousarray(X.T.reshape(2, 128, NPAD))
        xp = np.ascontiguousarray(X.reshape(NT, 128, N_FEAT).transpose(1, 0, 2))
        mask = np.zeros(NPAD, dtype=np.float32)
        mask[:NLOC] = 1.0
        mask = np.ascontiguousarray(mask.reshape(NT, 128).T)

        e = np.nonzero(owner == m)[0]
        cols = edge_col[e]
        order = np.argsort(cols, kind="stable")
        e = e[order]
        cols = cols[order]
        rows_l = (edge_row[e] - m * NLOC).astype(np.int32)
        vals = edge_val[e]
        chunk_of = cols // NLOC
        col_rel = (cols % NLOC).astype(np.int32)

        idxc_a = np.zeros((N_CALLS, 8, NI), dtype=np.int32)
        idxr_a = np.zeros((N_CALLS, 8, NI), dtype=np.int32)
        valc_a = np.zeros((N_CALLS, 8, NI), dtype=np.float32)
        for c in range(N_CHUNKS):
            ec = np.nonzero(chunk_of == c)[0]
            n_ec = ec.size
            assert n_ec <= SLOTS_PER_CHUNK, (m, c, n_ec)
            ccol = np.zeros(SLOTS_PER_CHUNK, dtype=np.int32)
            crow = np.zeros(SLOTS_PER_CHUNK, dtype=np.int32)
            cval = np.zeros(SLOTS_PER_CHUNK, dtype=np.float32)
            ccol[:n_ec] = col_rel[ec]
            crow[:n_ec] = rows_l[ec]
            cval[:n_ec] = vals[ec]
            base = c * CALLS_PER_CHUNK
            idxc_a[base:base + CALLS_PER_CHUNK] = ccol.reshape(
                CALLS_PER_CHUNK, 8, NI)
            idxr_a[base:base + CALLS_PER_CHUNK] = crow.reshape(
                CALLS_PER_CHUNK, 8, NI)
            valc_a[base:base + CALLS_PER_CHUNK] = cval.reshape(
                CALLS_PER_CHUNK, 8, NI)
        idxc_w = np.stack([_wrap16(idxc_a[k]) for k in range(N_CALLS)])
        idxr_w = np.stack([_wrap16(idxr_a[k]) for k in range(N_CALLS)])

        in_maps.append({
            "xt": xt, "xp": xp, "w_in": W, "b_in": bq, "maskin": mask,
            "idn": idn, "sel": sel, "onesin": ones, "onesrow": onesr,
            "idxc": idxc_w, "idxr": idxr_w, "valc": valc_a,
        })

    res = run_bass_kernel_spmd(nc, in_maps, core_ids=list(range(N_CORES)))

    assigns = np.concatenate(
        [res.results[m]["out_assign"][:NLOC] for m in range(N_CORES)], axis=0)
    fp = res.results[0]["out_feat"]
    losses = res.results[0]["out_loss"]
    return (fp, assigns, np.float32(losses[0, 0]), np.float32(losses[0, 1]))


# revision 8
# speedup vs baseline: 9.5239x; 1.0682x over previous
"""DMoN forward kernel on 8 TRN2 NeuronCores (Bass/Tile).

  S = softmax(X @ W + b)                       [N,16]  (assignments output)
  c = sum_n S[n];  P = S^T X;  features_pooled = selu(P / c[:,None])
  e2 = sum_e val_e (= 2*n_edges)
  m  = sum_e val_e * S[row_e]        (= S^T degrees)
  t1 = sum_e val_e * <S[row_e], S[col_e]>  (= trace(S^T A S))
  spectral_loss = -(t1 - |m|^2/e2)/e2
  collapse_loss = 0.1*(4*|c|/N - 1)

Nodes row-sharded 12500/core (padded 12544); edges sharded by row-owner,
sorted by col, bucketed into 8 col-chunks (one per owner rank). Per-edge
gathers via GPSIMD ap_gather on SBUF-resident S^T tables; one AllGather for
the global S^T table, one AllReduce for the small stats.
"""
import sys

sys.path.insert(0, "/opt/trn_rl_repo")

import numpy as np

N_CORES = 8
N_NODES = 100000
N_FEAT = 256
N_CLUSTERS = 16
NLOC = 12500
NPAD = 12544            # 98*128
NT = 98
NI = 1792               # ap_gather num_idxs per call
SLOTS_PER_CALL = 8 * NI
CALLS_PER_CHUNK = 4
N_CHUNKS = 8            # one col-chunk per owner rank (12544 nodes each)
SLOTS_PER_CHUNK = CALLS_PER_CHUNK * SLOTS_PER_CALL  # 57344
N_CALLS = N_CHUNKS * CALLS_PER_CHUNK                # 32
E_PAD = N_CHUNKS * SLOTS_PER_CHUNK                  # 458752

SELU_L = 1.0507009873554805
SELU_A = 1.6732632423543772


def _apply_tile_patches():
    import concourse.tile as tile_mod
    from concourse.vector_clock import ScopedClock, VectorClock

    def _patched_drain_and_barrier(self, tick_clock, wait_clock):
        nc = self.nc
        vc = tick_clock.global_clock
        for p in range(len(vc)):
            t = vc[p]
            if t > 0:
                vec = [0] * len(vc)
                vec[p] = t
                nop = nc.sync.nop()
                wait_clock.add_sem_waits(
                    nop.ins, ScopedClock({None: VectorClock(vec)}))
        nc.sync.drain()
        nc.all_engine_barrier()
        assert self.sems is not None
        popped = nc._tile_sem_poison_stack.pop()
        assert popped is self._sem_poison
        nc.clear_and_free_semaphores(list(self.sems.allocated().values()))
        nc.all_engine_barrier()

    tile_mod.TileContext._drain_and_barrier = _patched_drain_and_barrier


_FIX_CTR = [0]


def _fix_sync_waits(nc, max_waits=1):
    import concourse.mybir as mybir

    for bb in nc.m.functions[0].blocks:
        out, changed = [], False
        for inst in bb.instructions:
            si = inst.sync_info
            waits = list(si.on_wait) if si is not None and si.on_wait else []
            if len(waits) > max_waits:
                changed = True
                excess, keep = waits[:-max_waits], waits[-max_waits:]
                for i in range(0, len(excess), max_waits):
                    n = mybir.InstNoOp(
                        name=f"I-waitfix-{_FIX_CTR[0]}", ins=[], outs=[])
                    _FIX_CTR[0] += 1
                    n.engine = inst.engine
                    n.sync_info = mybir.SyncInfo(
                        on_wait=excess[i:i + max_waits], on_update=[])
                    out.append(n)
                inst.sync_info = mybir.SyncInfo(
                    on_wait=keep,
                    on_update=list(si.on_update) if si.on_update else [])
            out.append(inst)
        if changed:
            bb.instructions = out


_NC_CACHE = {}


def _build_program():
    if "nc" in _NC_CACHE:
        return _NC_CACHE["nc"]
    import concourse.bacc as bacc
    import concourse.bass as bass
    import concourse.mybir as mybir
    import concourse.tile as tile

    _apply_tile_patches()

    f32 = mybir.dt.float32
    i32 = mybir.dt.int32
    i16 = mybir.dt.int16
    AP = bass.AP

    nc = bacc.Bacc(None, target_bir_lowering=False)
    nc.num_devices = N_CORES

    xt = nc.dram_tensor("xt", [2, 128, NPAD], f32, kind="ExternalInput")
    xp = nc.dram_tensor("xp", [128, NT, N_FEAT], f32, kind="ExternalInput")
    w_in = nc.dram_tensor("w_in", [N_FEAT, N_CLUSTERS], f32, kind="ExternalInput")
    b_in = nc.dram_tensor("b_in", [1, N_CLUSTERS], f32, kind="ExternalInput")
    maskin = nc.dram_tensor("maskin", [128, NT], f32, kind="ExternalInput")
    idn = nc.dram_tensor("idn", [128, 128], f32, kind="ExternalInput")
    sel = nc.dram_tensor("sel", [128, N_CLUSTERS], f32, kind="ExternalInput")
    onesin = nc.dram_tensor("onesin", [128, 1], f32, kind="ExternalInput")
    onesrow = nc.dram_tensor("onesrow", [1, 128], f32, kind="ExternalInput")
    idxc = nc.dram_tensor("idxc", [N_CALLS, 128, NI // 16], i32, kind="ExternalInput")
    idxr = nc.dram_tensor("idxr", [N_CALLS, 128, NI // 16], i32, kind="ExternalInput")
    valc = nc.dram_tensor("valc", [N_CALLS, 8, NI], f32, kind="ExternalInput")

    out_assign = nc.dram_tensor("out_assign", [NPAD, N_CLUSTERS], f32,
                                kind="ExternalOutput")
    out_feat = nc.dram_tensor("out_feat", [N_CLUSTERS, N_FEAT], f32,
                              kind="ExternalOutput")
    out_loss = nc.dram_tensor("out_loss", [1, 2], f32, kind="ExternalOutput")

    STATS = 4224

    with tile.TileContext(nc) as tc:
        with tc.tile_pool(name="dram", bufs=1, space="DRAM") as dram, \
             tc.tile_pool(name="const", bufs=1) as const:

            cc_slt = dram.tile([16, NPAD], f32)
            ag_out = dram.tile([128, NPAD], f32, addr_space="Shared")
            st_in = dram.tile([1, STATS], f32)
            st_out = dram.tile([1, STATS], f32, addr_space="Shared")

            w_sb = const.tile([128, 2, N_CLUSTERS], f32)
            nc.sync.dma_start(w_sb[:],
                              w_in[:].rearrange("(h p) k -> p h k", p=128))
            b_sb = const.tile([1, N_CLUSTERS], f32)
            nc.sync.dma_start(b_sb[:], b_in[:])
            mask_sb = const.tile([128, NT], f32)
            nc.sync.dma_start(mask_sb[:], maskin[:])
            idn_sb = const.tile([128, 128], f32)
            nc.sync.dma_start(idn_sb[:], idn[:])
            sel_sb = const.tile([128, N_CLUSTERS], f32)
            nc.sync.dma_start(sel_sb[:], sel[:])
            ones_sb = const.tile([128, 1], f32)
            nc.sync.dma_start(ones_sb[:], onesin[:])
            onesr_sb = const.tile([1, 128], f32)
            nc.sync.dma_start(onesr_sb[:], onesrow[:])

            pt_acc = const.tile([128, 2 * N_CLUSTERS], f32)
            nc.vector.memset(pt_acc[:], 0.0)
            c_acc = const.tile([16, 1], f32)
            nc.vector.memset(c_acc[:], 0.0)

            accT = const.tile([128, N_CALLS], f32)
            accM = const.tile([128, N_CALLS], f32)
            accE = const.tile([128, N_CALLS], f32)

            # ---------------- phase A ----------------
            NCH, TPC = 7, NT // 7
            with tc.tile_pool(name="pA", bufs=2) as pA, \
                 tc.tile_pool(name="pAs", bufs=3) as pAs, \
                 tc.tile_pool(name="psT", bufs=2, space="PSUM") as psT:
                slt_sb = pA.tile([16, NPAD], f32, tag="slt", bufs=1)
                for ch in range(NCH):
                    n0 = ch * TPC * 128
                    xt_t = pA.tile([128, 2, TPC * 128], f32, tag="xt")
                    nc.sync.dma_start(
                        xt_t[:],
                        xt[:, :, n0:n0 + TPC * 128].rearrange("h p n -> p h n"))
                    xp_t = pA.tile([128, TPC, N_FEAT], f32, tag="xp")
                    nc.sync.dma_start(xp_t[:], xp[:, ch * TPC:(ch + 1) * TPC, :])
                    for t in range(TPC):
                        ti = ch * TPC + t
                        lg = psT.tile([128, N_CLUSTERS], f32, tag="lg")
                        for h in range(2):
                            nc.tensor.matmul(
                                lg[:], xt_t[:, h, t * 128:(t + 1) * 128],
                                w_sb[:, h, :], start=(h == 0), stop=False)
                        nc.tensor.matmul(lg[:], onesr_sb[:], b_sb[:],
                                         start=False, stop=True)
                        ex = pAs.tile([128, N_CLUSTERS], f32, tag="ex")
                        den = pAs.tile([128, 1], f32, tag="den")
                        nc.scalar.activation(ex[:], lg[:],
                                             mybir.ActivationFunctionType.Exp,
                                             accum_out=den[:])
                        rm = pAs.tile([128, 1], f32, tag="rm")
                        nc.vector.reciprocal(rm[:], den[:])
                        nc.vector.tensor_mul(rm[:], rm[:], mask_sb[:, ti:ti + 1])
                        s_t = pAs.tile([128, N_CLUSTERS], f32, tag="s_t")
                        nc.vector.tensor_scalar_mul(s_t[:], ex[:], rm[:])
                        nc.sync.dma_start(
                            out_assign[ti * 128:(ti + 1) * 128, :], s_t[:])
                        stp = psT.tile([16, 128], f32, tag="stp")
                        nc.tensor.transpose(stp[:], s_t[:], idn_sb[:])
                        nc.vector.tensor_copy(
                            slt_sb[:, ti * 128:(ti + 1) * 128], stp[:])
                        ptt = psT.tile([128, 2 * N_CLUSTERS], f32, tag="ptt")
                        for h in range(2):
                            nc.tensor.matmul(
                                ptt[:, h * N_CLUSTERS:(h + 1) * N_CLUSTERS],
                                xp_t[:, t, h * 128:(h + 1) * 128], s_t[:],
                                start=True, stop=True)
                        nc.vector.tensor_add(pt_acc[:], pt_acc[:], ptt[:])
                        ctt = psT.tile([16, 1], f32, tag="ctt")
                        nc.tensor.matmul(ctt[:], s_t[:], ones_sb[:],
                                         start=True, stop=True)
                        nc.vector.tensor_add(c_acc[:], c_acc[:], ctt[:])
                nc.sync.dma_start(cc_slt[:], slt_sb[:])

            nc.gpsimd.collective_compute(
                "AllGather", mybir.AluOpType.bypass,
                replica_groups=[list(range(N_CORES))],
                ins=[cc_slt[:]], outs=[ag_out[:]])

            # local-row table, replicated to all 8 q7 groups
            rt_sb = const.tile([128, NPAD], f32)
            nc.sync.dma_start(
                rt_sb[:],
                AP(cc_slt.tensor, cc_slt.offset,
                   [[0, 8], [NPAD, 16], [1, NPAD]]))

            # ---------------- edge phase ----------------
            with tc.tile_pool(name="pC", bufs=1) as pC, \
                 tc.tile_pool(name="pG", bufs=2) as pG:
                for chunk in range(N_CHUNKS):
                    ct_sb = pC.tile([128, NPAD], f32, tag="ct")
                    nc.sync.dma_start(
                        ct_sb[:],
                        AP(ag_out.tensor, ag_out.offset + 16 * chunk * NPAD,
                           [[0, 8], [NPAD, 16], [1, NPAD]]))
                    for s in range(CALLS_PER_CHUNK):
                        call = chunk * CALLS_PER_CHUNK + s
                        ic32 = pG.tile([128, NI // 16], i32, tag="ic32")
                        nc.sync.dma_start(ic32[:], idxc[call])
                        ic16 = pG.tile([128, NI // 16], i16, tag="ic16")
                        nc.vector.tensor_copy(ic16[:], ic32[:])
                        ir32 = pG.tile([128, NI // 16], i32, tag="ir32")
                        nc.sync.dma_start(ir32[:], idxr[call])
                        ir16 = pG.tile([128, NI // 16], i16, tag="ir16")
                        nc.vector.tensor_copy(ir16[:], ir32[:])
                        vr = pG.tile([128, NI], f32, tag="vr")
                        nc.sync.dma_start(
                            vr[:],
                            AP(valc[:].tensor, valc[:].offset + call * 8 * NI,
                               [[NI, 8], [0, 16], [1, NI]]))
                        gc = pG.tile([128, NI], f32, tag="gc")
                        nc.gpsimd.ap_gather(gc[:], ct_sb[:], ic16[:],
                                            channels=128, num_elems=NPAD,
                                            d=1, num_idxs=NI)
                        gr = pG.tile([128, NI], f32, tag="gr")
                        nc.gpsimd.ap_gather(gr[:], rt_sb[:], ir16[:],
                                            channels=128, num_elems=NPAD,
                                            d=1, num_idxs=NI)
                        grv = pG.tile([128, NI], f32, tag="grv")
                        nc.vector.tensor_mul(grv[:], gr[:], vr[:])
                        nc.vector.tensor_reduce(
                            accM[:, call:call + 1], grv[:],
                            axis=mybir.AxisListType.X, op=mybir.AluOpType.add)
                        nc.vector.tensor_mul(grv[:], grv[:], gc[:])
                        nc.vector.tensor_reduce(
                            accT[:, call:call + 1], grv[:],
                            axis=mybir.AxisListType.X, op=mybir.AluOpType.add)
                        nc.vector.tensor_reduce(
                            accE[:, call:call + 1], vr[:],
                            axis=mybir.AxisListType.X, op=mybir.AluOpType.add)

            # ---------------- reduce + final ----------------
            with tc.tile_pool(name="pF", bufs=1) as pF, \
                 tc.tile_pool(name="psF", bufs=1, space="PSUM") as psF:
                parts = pF.tile([128, 2], f32)
                nc.vector.tensor_reduce(parts[:, 0:1], accT[:],
                                        axis=mybir.AxisListType.X,
                                        op=mybir.AluOpType.add)
                nc.vector.tensor_reduce(parts[:, 1:2], accE[:],
                                        axis=mybir.AxisListType.X,
                                        op=mybir.AluOpType.add)
                mparts = pF.tile([128, 1], f32)
                nc.vector.tensor_reduce(mparts[:], accM[:],
                                        axis=mybir.AxisListType.X,
                                        op=mybir.AluOpType.add)
                te_ps = psF.tile([1, 2], f32)
                nc.tensor.matmul(te_ps[:], ones_sb[:], parts[:],
                                 start=True, stop=True)
                m_ps = psF.tile([16, 1], f32)
                nc.tensor.matmul(m_ps[:], sel_sb[:], mparts[:],
                                 start=True, stop=True)

                pt_sb = pt_acc
                c_sb0 = c_acc
                te_sb = pF.tile([1, 2], f32)
                nc.vector.tensor_copy(te_sb[:], te_ps[:])
                nc.vector.tensor_scalar_mul(te_sb[:, 1:2], te_sb[:, 1:2],
                                            1.0 / 16.0)
                m_sb0 = pF.tile([16, 1], f32)
                nc.vector.tensor_copy(m_sb0[:], m_ps[:])

                stt, sto = st_in.tensor, st_out.tensor
                nc.sync.dma_start(
                    AP(stt, st_in.offset, [[32, 128], [1, 32]]), pt_sb[:])
                nc.sync.dma_start(
                    AP(stt, st_in.offset + 4096, [[1, 16], [1, 1]]), c_sb0[:])
                nc.sync.dma_start(
                    AP(stt, st_in.offset + 4112, [[1, 16], [1, 1]]), m_sb0[:])
                nc.sync.dma_start(
                    AP(stt, st_in.offset + 4128, [[2, 1], [1, 2]]), te_sb[:])
                nc.gpsimd.collective_compute(
                    "AllReduce", mybir.AluOpType.add,
                    replica_groups=[list(range(N_CORES))],
                    ins=[st_in[:]], outs=[st_out[:]])

                ptr = pF.tile([128, 2 * N_CLUSTERS], f32)
                nc.sync.dma_start(
                    ptr[:], AP(sto, st_out.offset, [[32, 128], [1, 32]]))
                c_red = pF.tile([16, 1], f32)
                nc.sync.dma_start(
                    c_red[:], AP(sto, st_out.offset + 4096, [[1, 16], [1, 1]]))
                flat = pF.tile([1, 34], f32)
                nc.sync.dma_start(
                    flat[:], AP(sto, st_out.offset + 4096, [[34, 1], [1, 34]]))

                fp_ps = psF.tile([16, N_FEAT], f32)
                for h in range(2):
                    nc.tensor.transpose(
                        fp_ps[:, h * 128:(h + 1) * 128],
                        ptr[:, h * N_CLUSTERS:(h + 1) * N_CLUSTERS], idn_sb[:])
                rc = pF.tile([16, 1], f32)
                nc.vector.reciprocal(rc[:], c_red[:])
                fpd = pF.tile([16, N_FEAT], f32)
                nc.vector.tensor_scalar_mul(fpd[:], fp_ps[:], rc[:])
                pos = pF.tile([16, N_FEAT], f32)
                nc.vector.tensor_scalar_max(pos[:], fpd[:], 0.0)
                neg = pF.tile([16, N_FEAT], f32)
                nc.vector.tensor_scalar_min(neg[:], fpd[:], 0.0)
                en = pF.tile([16, N_FEAT], f32)
                nc.scalar.activation(en[:], neg[:],
                                     mybir.ActivationFunctionType.Exp)
                nc.vector.tensor_scalar(en[:], en[:], -1.0, SELU_L * SELU_A,
                                        op0=mybir.AluOpType.add,
                                        op1=mybir.AluOpType.mult)
                nc.vector.tensor_scalar_mul(pos[:], pos[:], SELU_L)
                nc.vector.tensor_add(pos[:], pos[:], en[:])
                nc.sync.dma_start(out_feat[:], pos[:])

                sq = pF.tile([1, 34], f32)
                nc.vector.tensor_mul(sq[:], flat[:], flat[:])
                cn = pF.tile([1, 4], f32)
                nc.vector.tensor_reduce(cn[:, 0:1], sq[:, 0:16],
                                        axis=mybir.AxisListType.X,
                                        op=mybir.AluOpType.add)
                nc.vector.tensor_reduce(cn[:, 1:2], sq[:, 16:32],
                                        axis=mybir.AxisListType.X,
                                        op=mybir.AluOpType.add)
                nc.vector.reciprocal(cn[:, 2:3], flat[:, 33:34])
                ls = pF.tile([1, 2], f32)
                nc.vector.tensor_mul(ls[:, 0:1], cn[:, 1:2], cn[:, 2:3])
                nc.vector.tensor_tensor(out=ls[:, 0:1], in0=ls[:, 0:1],
                                        in1=flat[:, 32:33],
                                        op=mybir.AluOpType.subtract)
                nc.vector.tensor_mul(ls[:, 0:1], ls[:, 0:1], cn[:, 2:3])
                nc.scalar.activation(cn[:, 3:4], cn[:, 0:1],
                                     mybir.ActivationFunctionType.Sqrt)
                nc.vector.tensor_scalar(
                    ls[:, 1:2], cn[:, 3:4],
                    float(np.sqrt(N_CLUSTERS)) / N_NODES, -1.0,
                    op0=mybir.AluOpType.mult, op1=mybir.AluOpType.add)
                nc.vector.tensor_scalar_mul(ls[:, 1:2], ls[:, 1:2], 0.1)
                nc.sync.dma_start(out_loss[:], ls[:])

    nc.finalize()
    _fix_sync_waits(nc)
    _NC_CACHE["nc"] = nc
    return nc


def _wrap16(idx_grp):
    out = np.zeros((128, NI // 16), dtype=np.int32)
    for g in range(8):
        out[g * 16:(g + 1) * 16, :] = idx_grp[g].reshape(NI // 16, 16).T
    return out


def kernel(features, W, b, edge_row, edge_col, edge_val):
    features = np.asarray(features, dtype=np.float32)
    W = np.asarray(W, dtype=np.float32)
    b = np.asarray(b, dtype=np.float32)
    edge_row = np.asarray(edge_row, dtype=np.int32)
    edge_col = np.asarray(edge_col, dtype=np.int32)
    edge_val = np.asarray(edge_val, dtype=np.float32)

    from concourse.bass_utils import run_bass_kernel_spmd

    nc = _build_program()

    idn = np.eye(128, dtype=np.float32)
    sel = np.zeros((128, 16), dtype=np.float32)
    for g in range(8):
        sel[g * 16:(g + 1) * 16] = np.eye(16, dtype=np.float32)
    ones = np.ones((128, 1), dtype=np.float32)
    onesr = np.ones((1, 128), dtype=np.float32)
    bq = b.reshape(1, 16).astype(np.float32)

    owner = edge_row // NLOC
    in_maps = []
    for m in range(N_CORES):
        X = np.zeros((NPAD, N_FEAT), dtype=np.float32)
        X[:NLOC] = features[m * NLOC:(m + 1) * NLOC]
        xt = np.ascontiguousarray(X.T.reshape(2, 128, NPAD))
        xp = np.ascontiguousarray(X.reshape(NT, 128, N_FEAT).transpose(1, 0, 2))
        mask = np.zeros(NPAD, dtype=np.float32)
        mask[:NLOC] = 1.0
        mask = np.ascontiguousarray(mask.reshape(NT, 128).T)

        e = np.nonzero(owner == m)[0]
        cols = edge_col[e]
        order = np.argsort(cols, kind="stable")
        e = e[order]
        cols = cols[order]
        rows_l = (edge_row[e] - m * NLOC).astype(np.int32)
        vals = edge_val[e]
        chunk_of = cols // NLOC
        col_rel = (cols % NLOC).astype(np.int32)

        idxc_a = np.zeros((N_CALLS, 8, NI), dtype=np.int32)
        idxr_a = np.zeros((N_CALLS, 8, NI), dtype=np.int32)
        valc_a = np.zeros((N_CALLS, 8, NI), dtype=np.float32)
        for c in range(N_CHUNKS):
            ec = np.nonzero(chunk_of == c)[0]
            n_ec = ec.size
            assert n_ec <= SLOTS_PER_CHUNK, (m, c, n_ec)
            ccol = np.zeros(SLOTS_PER_CHUNK, dtype=np.int32)
            crow = np.zeros(SLOTS_PER_CHUNK, dtype=np.int32)
            cval = np.zeros(SLOTS_PER_CHUNK, dtype=np.float32)
            ccol[:n_ec] = col_rel[ec]
            crow[:n_ec] = rows_l[ec]
            cval[:n_ec] = vals[ec]
            base = c * CALLS_PER_CHUNK
            idxc_a[base:base + CALLS_PER_CHUNK] = ccol.reshape(
                CALLS_PER_CHUNK, 8, NI)
            idxr_a[base:base + CALLS_PER_CHUNK] = crow.reshape(
                CALLS_PER_CHUNK, 8, NI)
            valc_a[base:base + CALLS_PER_CHUNK] = cval.reshape(
                CALLS_PER_CHUNK, 8, NI)
        idxc_w = np.stack([_wrap16(idxc_a[k]) for k in range(N_CALLS)])
        idxr_w = np.stack([_wrap16(idxr_a[k]) for k in range(N_CALLS)])

        in_maps.append({
            "xt": xt, "xp": xp, "w_in": W, "b_in": bq, "maskin": mask,
            "idn": idn, "sel": sel, "onesin": ones, "onesrow": onesr,
            "idxc": idxc_w, "idxr": idxr_w, "valc": valc_a,
        })

    res = run_bass_kernel_spmd(nc, in_maps, core_ids=list(range(N_CORES)))

    assigns = np.concatenate(
        [res.results[m]["out_assign"][:NLOC] for m in range(N_CORES)], axis=0)
    fp = res.results[0]["out_feat"]
    losses = res.results[0]["out_loss"]
    return (fp, assigns, np.float32(losses[0, 0]), np.float32(losses[0, 1]))
